# revision 3
# baseline (speedup 1.0000x reference)
"""Trainium2 Bass kernel for nn_Aggregator_13546326851764 (GNN message passing).

Strategy (8 NeuronCores, SPMD single program):
  - Shard KG edges by head range (aligned to head boundaries) and interactions
    by user-row range. Each core owns a disjoint output row range -> no
    all-reduce needed.
  - Host does INDEX-ONLY preprocessing: sorts edges by head, builds a fixed
    (tile x chunk) grid (64 segment rows per tile, fixed chunks-per-tile with
    dummy padding), and pre-gathers the input rows (entity_emb[tail],
    weight[type-1], entity_emb[col]) into linear streams in that order.
    This is data movement only - every floating-point operation happens on
    device. (Indirect/gather DMA hardware paths are non-functional in this
    environment, so the permutation is staged host-side and the device
    consumes linear streams.)
  - Device per chunk (128 edges): build one-hot(local_head) on DVE, multiply
    G by per-edge relation-weight rows (DVE), matmul one-hot.T @ data on PE
    accumulating a [64, 64] PSUM tile over the tile's chunks; divide by
    segment counts (entity side) and write out. Interaction side folds the
    per-edge value into the one-hot via a dual-op tensor_scalar.
  - Attention finisher (all on device): latent_new chain, user score chain
    (softmax over 8 factors), and user_agg = raw * (1 + score @ latent_new).
"""
import sys
import numpy as np

sys.path.insert(0, "/opt/trn_rl_repo")

import concourse.bass as bass
import concourse.bacc as bacc
import concourse.mybir as mybir
import concourse.tile as tile
from concourse.bass import AP
from concourse.bass_utils import run_bass_kernel_spmd
from concourse.masks import make_identity

# ---- problem constants (hardcoded per the contract) ----
N_USERS = 50000
N_ENTITIES = 100000
N_FACTORS = 8
N_RELATIONS = 17
EMB = 64
SLOPE = 0.2
N_CORES = 8
P = 128
H = 64                      # segment rows per tile
F32 = mybir.dt.float32
BF16 = mybir.dt.bfloat16

R_PAD_E = 13056             # padded entity rows per core (multiple of 128, > max shard)
R_PAD_U = 6272              # padded user rows per core (49*128, 50000/8 = 6250)
T_E = R_PAD_E // H          # entity tiles per core
T_U = R_PAD_U // H          # user tiles per core
DUMMY_LH = 127.0            # local head for padding edges (one-hot width 64 -> no hit)


# ----------------------------------------------------------------------------
# host-side preprocessing (pure index manipulation + data layout)
# ----------------------------------------------------------------------------

def _shard_bounds_by_segment(seg_sorted, n_seg, n_shards):
    """Split a sorted segment-id array into n_shards contiguous segment ranges
    with roughly equal edge counts. Returns segment boundaries [n_shards+1]."""
    n = seg_sorted.shape[0]
    bounds = [0]
    for c in range(1, n_shards):
        pos = (n * c) // n_shards
        bounds.append(int(seg_sorted[pos]))
    bounds.append(n_seg)
    # make strictly increasing (degenerate only in absurd distributions)
    for i in range(1, len(bounds)):
        if bounds[i] <= bounds[i - 1]:
            bounds[i] = bounds[i - 1] + 1
    return bounds


def _build_grid(seg_local, order, r_pad, cpt):
    """Given local segment ids (sorted ascending via `order`) build the fixed
    (tiles x cpt x 128) grid. Returns (perm, lh) where perm[i] is the source
    edge index feeding grid slot i (-1 for dummy) and lh[i] the local head
    within the tile (DUMMY_LH for dummies)."""
    n_tiles = r_pad // H
    slots = n_tiles * cpt * P
    perm = np.full(slots, -1, dtype=np.int64)
    lh = np.full(slots, DUMMY_LH, dtype=np.float32)
    seg_of_edge = seg_local[order]
    tile_of_edge = seg_of_edge // H
    # edges are sorted by segment => grouped by tile
    tile_starts = np.searchsorted(tile_of_edge, np.arange(n_tiles + 1))
    for t in range(n_tiles):
        s, e = tile_starts[t], tile_starts[t + 1]
        cnt = e - s
        assert cnt <= cpt * P, f"tile {t} has {cnt} edges > capacity {cpt * P}"
        base = t * cpt * P
        perm[base : base + cnt] = order[s:e]
        lh[base : base + cnt] = (seg_of_edge[s:e] - t * H).astype(np.float32)
    return perm, lh


def _to_tile_layout(arr, n_tiles, cpt, width):
    """[slots(=n_tiles*cpt*128), width] -> [n_tiles, 128, cpt, width]
    (partition-major within a tile so one DMA per tile works)."""
    a = arr.reshape(n_tiles, cpt, P, width)
    return np.ascontiguousarray(a.transpose(0, 2, 1, 3))


def preprocess(inputs):
    ent = np.asarray(inputs["entity_emb"], np.float32)
    usr = np.asarray(inputs["user_emb"], np.float32)
    weight = np.asarray(inputs["weight"], np.float32)
    head = np.asarray(inputs["edge_index"][0]).astype(np.int64)
    tail = np.asarray(inputs["edge_index"][1]).astype(np.int64)
    etype = np.asarray(inputs["edge_type"]).astype(np.int64)
    irow = np.asarray(inputs["interact_rows"]).astype(np.int64)
    icol = np.asarray(inputs["interact_cols"]).astype(np.int64)
    ival = np.asarray(inputs["interact_vals"], np.float32)

    # ---- KG side: shard by head range ----
    ho = np.argsort(head, kind="stable")
    head_s = head[ho]
    hb = _shard_bounds_by_segment(head_s, N_ENTITIES, N_CORES)
    # ---- interact side: shard by fixed user ranges ----
    ub = [min(6250 * c, N_USERS) for c in range(N_CORES + 1)]

    # fixed chunks-per-tile, computed globally so the SPMD program is shared
    cpt_e, cpt_u = 0, 0
    core_edge_idx, core_nnz_idx = [], []
    for c in range(N_CORES):
        m = (head >= hb[c]) & (head < hb[c + 1])
        idx = np.nonzero(m)[0]
        loc = head[idx] - hb[c]
        o = np.argsort(loc, kind="stable")
        core_edge_idx.append((idx, loc, o))
        tcounts = np.bincount(loc[o] // H, minlength=R_PAD_E // H)
        cpt_e = max(cpt_e, int(np.ceil(tcounts.max() / P)))

        m2 = (irow >= ub[c]) & (irow < ub[c + 1])
        idx2 = np.nonzero(m2)[0]
        loc2 = irow[idx2] - ub[c]
        o2 = np.argsort(loc2, kind="stable")
        core_nnz_idx.append((idx2, loc2, o2))
        tcounts2 = np.bincount(loc2[o2] // H, minlength=R_PAD_U // H)
        cpt_u = max(cpt_u, int(np.ceil(tcounts2.max() / P)))

    zero_row = np.zeros((1, EMB), np.float32)
    ent_x = np.vstack([ent, zero_row])             # row N_ENTITIES = zeros for dummies
    w_x = np.vstack([weight, zero_row])            # row 16 = zeros for dummies

    in_maps = []
    meta = {"hb": hb, "ub": ub, "cpt_e": cpt_e, "cpt_u": cpt_u}
    for c in range(N_CORES):
        idx, loc, o = core_edge_idx[c]
        perm, lh = _build_grid(loc, o, R_PAD_E, cpt_e)
        src = np.where(perm >= 0, idx[np.clip(perm, 0, None)], -1)
        tails = np.where(src >= 0, tail[np.clip(src, 0, None)], N_ENTITIES)
        types = np.where(src >= 0, etype[np.clip(src, 0, None)] - 1, N_RELATIONS - 1)
        kg_g = _to_tile_layout(ent_x[tails], T_E, cpt_e, EMB)
        kg_w = _to_tile_layout(w_x[types].astype(np.float32), T_E, cpt_e, EMB).astype(np.float32)
        kg_lh = _to_tile_layout(lh[:, None], T_E, cpt_e, 1)[..., 0]     # [T_E,128,cpt]
        cnt = np.bincount(loc, minlength=R_PAD_E).astype(np.float32)[:R_PAD_E]
        kg_cnt = cnt.reshape(T_E // 2, P).T.copy()                      # [128, T_E//2]

        idx2, loc2, o2 = core_nnz_idx[c]
        perm2, lh2 = _build_grid(loc2, o2, R_PAD_U, cpt_u)
        src2 = np.where(perm2 >= 0, idx2[np.clip(perm2, 0, None)], -1)
        cols = np.where(src2 >= 0, icol[np.clip(src2, 0, None)], N_ENTITIES)
        vals = np.where(src2 >= 0, ival[np.clip(src2, 0, None)], 0.0).astype(np.float32)
        it_g = _to_tile_layout(ent_x[cols], T_U, cpt_u, EMB)
        it_lh = _to_tile_layout(lh2[:, None], T_U, cpt_u, 1)[..., 0]
        it_v = _to_tile_layout(vals[:, None], T_U, cpt_u, 1)[..., 0]

        ue = np.zeros((R_PAD_U, EMB), np.float32)
        ue[: ub[c + 1] - ub[c]] = usr[ub[c] : ub[c + 1]]

        in_maps.append({
            "kg_g": kg_g, "kg_w": kg_w.astype(np.float32), "kg_lh": kg_lh,
            "kg_cnt": kg_cnt,
            "it_g": it_g, "it_lh": it_lh, "it_v": it_v,
            "ueT": np.ascontiguousarray(ue.T),
            "latT": np.ascontiguousarray(np.asarray(inputs["latent_emb"], np.float32).T),
            "wgt": weight,
            "wgtT": np.ascontiguousarray(weight.T),
            "W1T": np.ascontiguousarray(np.asarray(inputs["W1"], np.float32).T),
            "W2T": np.ascontiguousarray(np.asarray(inputs["W2"], np.float32).T),
            "WuaT": np.ascontiguousarray(np.asarray(inputs["W_user_att"], np.float32).T),
            "WwaT": np.ascontiguousarray(np.asarray(inputs["W_weight_att"], np.float32).T),
            "b1": np.asarray(inputs["b1"], np.float32)[None, :],
            "b2": np.asarray(inputs["b2"], np.float32)[None, :],
            "bua": np.asarray(inputs["b_user_att"], np.float32)[None, :],
            "bwa": np.asarray(inputs["b_weight_att"], np.float32)[None, :],
        })
    return in_maps, meta


# ----------------------------------------------------------------------------
# device program
# ----------------------------------------------------------------------------

def _bcast_mid(ap_2d, mid, width):
    """[P, width] AP -> [P, mid, width] with stride-0 middle dim."""
    return AP(ap_2d.tensor, ap_2d.offset, [ap_2d.ap[0], [0, mid], ap_2d.ap[1]])


def build_program(cpt_e, cpt_u, reps=1):
    nc = bacc.Bacc(None, target_bir_lowering=False)
    dp = nc.declare_dram_parameter
    kg_g = dp("kg_g", [T_E, P, cpt_e, EMB], F32, isOutput=False)
    kg_w = dp("kg_w", [T_E, P, cpt_e, EMB], F32, isOutput=False)
    kg_lh = dp("kg_lh", [T_E, P, cpt_e], F32, isOutput=False)
    kg_cnt = dp("kg_cnt", [P, T_E // 2], F32, isOutput=False)
    it_g = dp("it_g", [T_U, P, cpt_u, EMB], F32, isOutput=False)
    it_lh = dp("it_lh", [T_U, P, cpt_u], F32, isOutput=False)
    it_v = dp("it_v", [T_U, P, cpt_u], F32, isOutput=False)
    ueT = dp("ueT", [EMB, R_PAD_U], F32, isOutput=False)
    latT = dp("latT", [EMB, N_FACTORS], F32, isOutput=False)
    wgt = dp("wgt", [N_RELATIONS - 1, EMB], F32, isOutput=False)
    wgtT = dp("wgtT", [EMB, N_RELATIONS - 1], F32, isOutput=False)
    W1T = dp("W1T", [EMB, EMB], F32, isOutput=False)
    W2T = dp("W2T", [EMB, EMB], F32, isOutput=False)
    WuaT = dp("WuaT", [N_FACTORS, N_FACTORS], F32, isOutput=False)
    WwaT = dp("WwaT", [N_RELATIONS - 1, N_RELATIONS - 1], F32, isOutput=False)
    b1 = dp("b1", [1, EMB], F32, isOutput=False)
    b2 = dp("b2", [1, EMB], F32, isOutput=False)
    bua = dp("bua", [1, N_FACTORS], F32, isOutput=False)
    bwa = dp("bwa", [1, N_RELATIONS - 1], F32, isOutput=False)
    ent_out = dp("ent_out", [R_PAD_E, EMB], F32, isOutput=True)
    usr_out = dp("usr_out", [R_PAD_U, EMB], F32, isOutput=True)
    lat_out = dp("lat_out", [N_FACTORS, EMB], F32, isOutput=True)
    usr_raw = nc.dram_tensor("usr_raw", [R_PAD_U, EMB], F32)

    AT = mybir.ActivationFunctionType
    OP = mybir.AluOpType

    with tile.TileContext(nc) as tc:
        with (
            tc.tile_pool(name="const", bufs=1) as cons,
            tc.tile_pool(name="stream", bufs=3) as st,
            tc.tile_pool(name="work", bufs=3) as wk,
            tc.tile_pool(name="ps", bufs=2, space="PSUM") as ps,
            tc.tile_pool(name="fin", bufs=3) as fin,
            tc.tile_pool(name="psf", bufs=2, space="PSUM") as psf,
        ):
            # constants
            ident = cons.tile([P, P], F32)
            make_identity(nc, ident[:])
            iota_i = cons.tile([P, H], mybir.dt.int32)
            nc.gpsimd.iota(iota_i[:], pattern=[[1, H]], base=0, channel_multiplier=0)
            iota = cons.tile([P, H], F32)
            nc.vector.tensor_copy(out=iota[:], in_=iota_i[:])
            invc = cons.tile([P, T_E // 2], F32)
            cntt = cons.tile([P, T_E // 2], F32)
            nc.sync.dma_start(out=cntt[:], in_=kg_cnt[:, :])
            nc.vector.tensor_scalar_max(out=cntt[:], in0=cntt[:], scalar1=1.0)
            nc.vector.reciprocal(out=invc[:], in_=cntt[:])

            w1t = cons.tile([EMB, EMB], F32, tag="w1t")
            w2t = cons.tile([EMB, EMB], F32, tag="w2t")
            wuat = cons.tile([N_FACTORS, N_FACTORS], F32, tag="wuat")
            wwat = cons.tile([N_RELATIONS - 1, N_RELATIONS - 1], F32, tag="wwat")
            latTs = cons.tile([EMB, N_FACTORS], F32, tag="latTs")
            wgts = cons.tile([N_RELATIONS - 1, EMB], F32, tag="wgts")
            wgtTs = cons.tile([EMB, N_RELATIONS - 1], F32, tag="wgtTs")
            b1r = cons.tile([P, EMB], F32, tag="b1r")
            b2r = cons.tile([P, EMB], F32, tag="b2r")
            buar = cons.tile([P, N_FACTORS], F32, tag="buar")
            bwar = cons.tile([P, N_RELATIONS - 1], F32, tag="bwar")
            nc.sync.dma_start(out=w1t[:], in_=W1T[:, :])
            nc.sync.dma_start(out=w2t[:], in_=W2T[:, :])
            nc.sync.dma_start(out=wuat[:], in_=WuaT[:, :])
            nc.sync.dma_start(out=wwat[:], in_=WwaT[:, :])
            nc.sync.dma_start(out=latTs[:], in_=latT[:, :])
            nc.sync.dma_start(out=wgts[:], in_=wgt[:, :])
            nc.sync.dma_start(out=wgtTs[:], in_=wgtT[:, :])
            for dst, srcp in ((b1r, b1), (b2r, b2), (buar, bua), (bwar, bwa)):
                wdt = dst.shape[-1]
                ap0 = srcp[:, :]
                nc.sync.dma_start(out=dst[:], in_=AP(ap0.tensor, 0, [[0, P], [1, wdt]]))

            for _rep in range(reps):
                # ============ KG segment mean ============
                for t in range(T_E):
                    g = st.tile([P, cpt_e, EMB], F32, tag="kg_g")
                    w = st.tile([P, cpt_e, EMB], F32, tag="kg_w")
                    lh = st.tile([P, cpt_e], F32, tag="kg_lh")
                    nc.sync.dma_start(out=g[:], in_=kg_g[t])
                    nc.sync.dma_start(out=w[:], in_=kg_w[t])
                    nc.sync.dma_start(out=lh[:], in_=kg_lh[t])
                    gw = wk.tile([P, cpt_e, EMB], F32, tag="gw")
                    nc.vector.tensor_tensor(out=gw[:], in0=g[:], in1=w[:], op=OP.mult)
                    acc = ps.tile([H, EMB], F32, tag="acc")
                    oh = wk.tile([P, cpt_e, H], F32, tag="oh")
                    for k in range(cpt_e):
                        nc.vector.tensor_scalar(
                            out=oh[:, k, :], in0=iota[:], scalar1=lh[:, k : k + 1],
                            scalar2=None, op0=OP.is_equal)
                        nc.tensor.matmul(
                            acc[:], lhsT=oh[:, k, :], rhs=gw[:, k, :],
                            start=(k == 0), stop=(k == cpt_e - 1))
                    res = fin.tile([H, EMB], F32, tag="eres")
                    col = t // 2
                    half = (t % 2) * H
                    nc.vector.tensor_scalar(
                        out=res[:], in0=acc[:],
                        scalar1=invc[half : half + H, col : col + 1],
                        scalar2=None, op0=OP.mult)
                    nc.sync.dma_start(out=ent_out[t * H : (t + 1) * H, :], in_=res[:])

                # ============ interaction segment sum ============
                for t in range(T_U):
                    g = st.tile([P, cpt_u, EMB], F32, tag="it_g")
                    lh = st.tile([P, cpt_u], F32, tag="it_lh")
                    vv = st.tile([P, cpt_u], F32, tag="it_v")
                    nc.sync.dma_start(out=g[:], in_=it_g[t])
                    nc.sync.dma_start(out=lh[:], in_=it_lh[t])
                    nc.sync.dma_start(out=vv[:], in_=it_v[t])
                    acc = ps.tile([H, EMB], F32, tag="acc")
                    oh = wk.tile([P, cpt_u, H], F32, tag="ohu")
                    for k in range(cpt_u):
                        nc.vector.tensor_scalar(
                            out=oh[:, k, :], in0=iota[:], scalar1=lh[:, k : k + 1],
                            scalar2=vv[:, k : k + 1], op0=OP.is_equal, op1=OP.mult)
                        nc.tensor.matmul(
                            acc[:], lhsT=oh[:, k, :], rhs=g[:, k, :],
                            start=(k == 0), stop=(k == cpt_u - 1))
                    res = fin.tile([H, EMB], F32, tag="ures")
                    nc.vector.tensor_copy(out=res[:], in_=acc[:])
                    nc.sync.dma_start(out=usr_raw[t * H : (t + 1) * H, :], in_=res[:])

                # ============ latent_new chain (tiny, replicated) ============

                # L2 = latent @ W2.T + b2   [8, 64]
                pl = psf.tile([N_FACTORS, EMB], F32, tag="pst")
                nc.tensor.matmul(pl[:], lhsT=latTs[:], rhs=w2t[:], start=True, stop=True)
                l2 = fin.tile([N_FACTORS, EMB], F32, tag="l2")
                nc.vector.tensor_tensor(out=l2[:], in0=pl[:], in1=b2r[:N_FACTORS, :], op=OP.add)
                # Wv = weight @ W2.T + b2   [16, 64]
                pw = psf.tile([N_RELATIONS - 1, EMB], F32, tag="pst")
                nc.tensor.matmul(pw[:], lhsT=wgtTs[:], rhs=w2t[:], start=True, stop=True)
                wv = fin.tile([N_RELATIONS - 1, EMB], F32, tag="wv")
                nc.vector.tensor_tensor(out=wv[:], in0=pw[:], in1=b2r[: N_RELATIONS - 1, :], op=OP.add)
                # score_rp = L2 @ Wv.T  [8, 16]: need L2T [64, 8] and WvT [64, 16]
                pt = psf.tile([EMB, N_FACTORS], F32, tag="pst")
                nc.tensor.transpose(out=pt[:], in_=l2[:], identity=ident[:N_FACTORS, :N_FACTORS])
                l2t = fin.tile([EMB, N_FACTORS], F32, tag="l2t")
                nc.vector.tensor_copy(out=l2t[:], in_=pt[:])
                pt2 = psf.tile([EMB, N_RELATIONS - 1], F32, tag="pst")
                nc.tensor.transpose(out=pt2[:], in_=wv[:], identity=ident[:N_RELATIONS - 1, :N_RELATIONS - 1])
                wvt = fin.tile([EMB, N_RELATIONS - 1], F32, tag="wvt")
                nc.vector.tensor_copy(out=wvt[:], in_=pt2[:])
                prp = psf.tile([N_FACTORS, N_RELATIONS - 1], F32, tag="pst")
                nc.tensor.matmul(prp[:], lhsT=l2t[:], rhs=wvt[:], start=True, stop=True)
                srp = fin.tile([N_FACTORS, N_RELATIONS - 1], F32, tag="srp")
                nc.vector.tensor_copy(out=srp[:], in_=prp[:])
                # att = lrelu(score_rp @ Wwa.T + bwa)  [8, 16]
                pt3 = psf.tile([N_RELATIONS - 1, N_FACTORS], F32, tag="pst")
                nc.tensor.transpose(out=pt3[:], in_=srp[:], identity=ident[:N_FACTORS, :N_FACTORS])
                srpt = fin.tile([N_RELATIONS - 1, N_FACTORS], F32, tag="srpt")
                nc.vector.tensor_copy(out=srpt[:], in_=pt3[:])
                par = psf.tile([N_FACTORS, N_RELATIONS - 1], F32, tag="pst")
                nc.tensor.matmul(par[:], lhsT=srpt[:], rhs=wwat[:], start=True, stop=True)
                att = fin.tile([N_FACTORS, N_RELATIONS - 1], F32, tag="att")
                nc.vector.tensor_tensor(out=att[:], in0=par[:], in1=bwar[:N_FACTORS, :], op=OP.add)
                lr = fin.tile([N_FACTORS, N_RELATIONS - 1], F32, tag="lr")
                nc.vector.tensor_scalar_mul(out=lr[:], in0=att[:], scalar1=SLOPE)
                nc.vector.tensor_tensor(out=att[:], in0=att[:], in1=lr[:], op=OP.max)
                # softmax over 16
                mx = fin.tile([N_FACTORS, 1], F32, tag="mx")
                nc.vector.reduce_max(out=mx[:], in_=att[:], axis=mybir.AxisListType.X)
                nc.vector.tensor_scalar(out=att[:], in0=att[:], scalar1=mx[:, :1],
                                        scalar2=None, op0=OP.subtract)
                nc.scalar.activation(out=att[:], in_=att[:], func=AT.Exp)
                sm = fin.tile([N_FACTORS, 1], F32, tag="sm")
                nc.vector.reduce_sum(out=sm[:], in_=att[:], axis=mybir.AxisListType.X)
                nc.vector.reciprocal(out=sm[:], in_=sm[:])
                nc.vector.tensor_scalar(out=att[:], in0=att[:], scalar1=sm[:, :1],
                                        scalar2=None, op0=OP.mult)
                # latent_new = att @ weight  [8, 64]
                pt4 = psf.tile([N_RELATIONS - 1, N_FACTORS], F32, tag="pst")
                nc.tensor.transpose(out=pt4[:], in_=att[:], identity=ident[:N_FACTORS, :N_FACTORS])
                attt = fin.tile([N_RELATIONS - 1, N_FACTORS], F32, tag="attt")
                nc.vector.tensor_copy(out=attt[:], in_=pt4[:])
                pln = psf.tile([N_FACTORS, EMB], F32, tag="pst")
                nc.tensor.matmul(pln[:], lhsT=attt[:], rhs=wgts[:], start=True, stop=True)
                latnew = cons.tile([N_FACTORS, EMB], F32, tag="latnew")
                nc.vector.tensor_copy(out=latnew[:], in_=pln[:])
                nc.sync.dma_start(out=lat_out[:, :], in_=latnew[:])
                # A_lin = latent @ W1.T + b1 -> transposed [64, 8]
                pa = psf.tile([N_FACTORS, EMB], F32, tag="pst")
                nc.tensor.matmul(pa[:], lhsT=latTs[:], rhs=w1t[:], start=True, stop=True)
                alin = fin.tile([N_FACTORS, EMB], F32, tag="alin")
                nc.vector.tensor_tensor(out=alin[:], in0=pa[:], in1=b1r[:N_FACTORS, :], op=OP.add)
                pat = psf.tile([EMB, N_FACTORS], F32, tag="pst")
                nc.tensor.transpose(out=pat[:], in_=alin[:], identity=ident[:N_FACTORS, :N_FACTORS])
                alint = cons.tile([EMB, N_FACTORS], F32, tag="alint")
                nc.vector.tensor_copy(out=alint[:], in_=pat[:])

                # ============ user attention + residual update ============
                for t in range(R_PAD_U // P):
                    u0 = t * P
                    # B = ue @ W1.T + b1  [128, 64]
                    pb = psf.tile([P, EMB], F32, tag="pst")
                    uslice = st.tile([EMB, P], F32, tag="uslice")
                    nc.sync.dma_start(out=uslice[:], in_=ueT[:, u0 : u0 + P])
                    nc.tensor.matmul(pb[:], lhsT=uslice[:], rhs=w1t[:], start=True, stop=True)
                    bt_ = wk.tile([P, EMB], F32, tag="bt_")
                    nc.vector.tensor_tensor(out=bt_[:], in0=pb[:], in1=b1r[:], op=OP.add)
                    # BT [64, 128]
                    pbt = psf.tile([EMB, P], F32, tag="pst")
                    nc.tensor.transpose(out=pbt[:], in_=bt_[:], identity=ident[:])
                    btt = wk.tile([EMB, P], F32, tag="btt")
                    nc.vector.tensor_copy(out=btt[:], in_=pbt[:])
                    # score_ = B @ A_lin.T [128, 8]
                    psc = psf.tile([P, N_FACTORS], F32, tag="pst")
                    nc.tensor.matmul(psc[:], lhsT=btt[:], rhs=alint[:], start=True, stop=True)
                    sc = wk.tile([P, N_FACTORS], F32, tag="sc")
                    nc.vector.tensor_copy(out=sc[:], in_=psc[:])
                    # att_u = lrelu(score_ @ Wua.T + bua)
                    psct = psf.tile([N_FACTORS, P], F32, tag="pst")
                    nc.tensor.transpose(out=psct[:], in_=sc[:], identity=ident[:])
                    sct = wk.tile([N_FACTORS, P], F32, tag="sct")
                    nc.vector.tensor_copy(out=sct[:], in_=psct[:])
                    pau = psf.tile([P, N_FACTORS], F32, tag="pst")
                    nc.tensor.matmul(pau[:], lhsT=sct[:], rhs=wuat[:], start=True, stop=True)
                    au = wk.tile([P, N_FACTORS], F32, tag="au")
                    nc.vector.tensor_tensor(out=au[:], in0=pau[:], in1=buar[:], op=OP.add)
                    lru = wk.tile([P, N_FACTORS], F32, tag="lru")
                    nc.vector.tensor_scalar_mul(out=lru[:], in0=au[:], scalar1=SLOPE)
                    nc.vector.tensor_tensor(out=au[:], in0=au[:], in1=lru[:], op=OP.max)
                    mxu = wk.tile([P, 1], F32, tag="mxu")
                    nc.vector.reduce_max(out=mxu[:], in_=au[:], axis=mybir.AxisListType.X)
                    nc.vector.tensor_scalar(out=au[:], in0=au[:], scalar1=mxu[:, :1],
                                            scalar2=None, op0=OP.subtract)
                    nc.scalar.activation(out=au[:], in_=au[:], func=AT.Exp)
                    smu = wk.tile([P, 1], F32, tag="smu")
                    nc.vector.reduce_sum(out=smu[:], in_=au[:], axis=mybir.AxisListType.X)
                    nc.vector.reciprocal(out=smu[:], in_=smu[:])
                    nc.vector.tensor_scalar(out=au[:], in0=au[:], scalar1=smu[:, :1],
                                            scalar2=None, op0=OP.mult)
                    # um = score @ latent_new [128, 64]
                    paut = psf.tile([N_FACTORS, P], F32, tag="pst")
                    nc.tensor.transpose(out=paut[:], in_=au[:], identity=ident[:])
                    aut = wk.tile([N_FACTORS, P], F32, tag="aut")
                    nc.vector.tensor_copy(out=aut[:], in_=paut[:])
                    pum = psf.tile([P, EMB], F32, tag="pst")
                    nc.tensor.matmul(pum[:], lhsT=aut[:], rhs=latnew[:], start=True, stop=True)
                    um = wk.tile([P, EMB], F32, tag="um")
                    nc.vector.tensor_scalar_add(out=um[:], in0=pum[:], scalar1=1.0)
                    raw = wk.tile([P, EMB], F32, tag="raw")
                    nc.sync.dma_start(out=raw[:], in_=usr_raw[u0 : u0 + P, :])
                    nc.vector.tensor_tensor(out=raw[:], in0=raw[:], in1=um[:], op=OP.mult)
                    nc.sync.dma_start(out=usr_out[u0 : u0 + P, :], in_=raw[:])
    nc.finalize()
    return nc


_PROGRAM_CACHE = {}


def kernel(**inputs):
    in_maps, meta = preprocess(inputs)
    key = (meta["cpt_e"], meta["cpt_u"])
    if key not in _PROGRAM_CACHE:
        _PROGRAM_CACHE[key] = build_program(*key)
    nc = _PROGRAM_CACHE[key]
    res = run_bass_kernel_spmd(nc, in_maps, list(range(N_CORES)))
    hb, ub = meta["hb"], meta["ub"]
    ent = np.empty((N_ENTITIES, EMB), np.float32)
    usr = np.empty((N_USERS, EMB), np.float32)
    for c in range(N_CORES):
        ent[hb[c] : hb[c + 1]] = res.results[c]["ent_out"][: hb[c + 1] - hb[c]]
        usr[ub[c] : ub[c + 1]] = res.results[c]["usr_out"][: ub[c + 1] - ub[c]]
    lat = res.results[0]["lat_out"]
    return ent, usr, lat
